# revision 1
# baseline (speedup 1.0000x reference)
"""Causal multi-head attention with RoPE for Trainium2, 8-core SPMD.

Problem: B=2, S=2048, D_MODEL=1024, H=16, HD=64, causal softmax(QK^T/8)V
with interleaved-pair RoPE on q/k, projections Wq/Wk/Wv/Wo.

Sharding (host side): batch x head-group. Core c handles batch b=c//4 and
head group g=c%4 (heads 4g..4g+3, a 256-wide slice of the projection dims).
Each core computes a full [S, D_MODEL] partial of the output (its head
group's contribution through Wo); host sums 4 partials per batch.

Device layout strategy (all matmuls bf16, fp32 accumulate):
 - host passes x[b].T so the d-contraction sits on SBUF partitions
 - Q,K projected in [s, o] layout -> RoPE on DVE along free dim (pairs are
   adjacent columns) -> bf16 -> DMA-transposed (XBAR, bf16) into [o, s]
 - scores^T[k, q] = Kt.T @ Qt per 128-key block (K=64 contraction); the
   two heads of a pair are issued to PE row groups 0/64 (tile_position)
   and run concurrently. Blocks land in wide PSUM tiles, one Exp per wide
   tile (ACT amortizes its 352-cycle fixed cost), causal-masked by
   multiplying the diagonal 128x128 block; q-columns below the diagonal
   are never computed or consumed
 - PV: lhsT = [V | 1] per key block (M=65) so row 64 of the PSUM output
   accumulates the softmax denominator for free; DVE normalizes
 - o_proj consumes the attention output, PSUM is DMA'd straight to DRAM
"""

import numpy as np
import ml_dtypes

B, S, D, H = 2, 2048, 1024, 16
HD = 64
NCORES = 8
HEADS_PER_CORE = 4
GDIM = HEADS_PER_CORE * HD          # 256 projection cols per core
SB = S // 128                        # 16 s-tiles
KD = D // 128                        # 8 k-tiles over d
QCHUNK = 512
NQC = S // QCHUNK                    # 4 q-chunks
WIDE = 1024                          # wide scores psum tile (2 banks)

_BF16 = ml_dtypes.bfloat16
_cache = {}


def _build(use_rope: bool, reps: int = 1, timing: bool = False, phases=(1, 2, 3)):
    import concourse.bass as bass
    import concourse.mybir as mybir
    import concourse.tile as tile
    from concourse import bacc

    F32 = mybir.dt.float32
    BF16 = mybir.dt.bfloat16
    EXP = mybir.ActivationFunctionType.Exp

    nc = bacc.Bacc(None, target_bir_lowering=False)

    xt_d = nc.dram_tensor("xt", [D, S], BF16, kind="ExternalInput")
    wqk_d = nc.dram_tensor("wqk", [D, 2 * GDIM], BF16, kind="ExternalInput")
    wv_d = nc.dram_tensor("wv", [D, GDIM], BF16, kind="ExternalInput")
    wo_d = nc.dram_tensor("wo", [GDIM, D], BF16, kind="ExternalInput")
    cos_d = nc.dram_tensor("cos8", [S, 256], BF16, kind="ExternalInput")
    sin_d = nc.dram_tensor("sin8", [S, 256], BF16, kind="ExternalInput")
    mask_d = nc.dram_tensor("maskT", [128, 128], BF16, kind="ExternalInput")
    if timing:
        # timing builds: full-size output stays on device (internal DRAM);
        # tiny external output avoids 64MB host transfers per timed call
        out_d = nc.dram_tensor("oscratch", [S, D], F32)
        out_small = nc.dram_tensor("out", [128, 512], F32, kind="ExternalOutput")
    else:
        out_d = nc.dram_tensor("out", [S, D], F32, kind="ExternalOutput")
        out_small = None

    with tile.TileContext(nc) as tc:
        with tc.tile_pool(name="big", bufs=1) as big, \
             tc.tile_pool(name="work", bufs=3) as work, \
             tc.tile_pool(name="ropet", bufs=4) as ropet, \
             tc.tile_pool(name="pex", bufs=4) as pex:
            # ---- resident tensors ----
            xt = big.tile([128, KD, S], BF16)
            nc.sync.dma_start(xt[:], xt_d.rearrange("(k p) s -> p k s", p=128))
            wqk = big.tile([128, KD, 2 * GDIM], BF16)
            nc.sync.dma_start(wqk[:], wqk_d.rearrange("(k p) o -> p k o", p=128))
            wv = big.tile([128, KD, GDIM], BF16)
            nc.sync.dma_start(wv[:], wv_d.rearrange("(k p) o -> p k o", p=128))
            wo = big.tile([128, 2, D], BF16)
            nc.sync.dma_start(wo[:], wo_d.rearrange("(k p) o -> p k o", p=128))
            maskT = big.tile([128, 128], BF16)
            nc.sync.dma_start(maskT[:], mask_d[:])
            if use_rope:
                cos8 = big.tile([128, SB, 256], BF16)
                nc.sync.dma_start(cos8[:], cos_d.rearrange("(m p) f -> p m f", p=128))
                sin8 = big.tile([128, SB, 256], BF16)
                nc.sync.dma_start(sin8[:], sin_d.rearrange("(m p) f -> p m f", p=128))

            # attention-side resident tiles
            qkt = [big.tile([128, S], BF16, tag=f"qkt{i}", name=f"qkt{i}")
                   for i in range(4)]
            # qkt[0]: Qt heads 0-1, qkt[1]: Qt heads 2-3, qkt[2]: Kt 0-1, qkt[3]: Kt 2-3
            vsb = big.tile([128, SB, HEADS_PER_CORE * 65], BF16)
            yt2 = [big.tile([128, S], BF16, tag=f"yt2{i}", name=f"yt2{i}")
                   for i in range(2)]

            for _rep in range(reps):
                # PSUM plan: sc(4 banks)+yt(2)+pp(2) live together; pp closes
                # after phase 1 and op(2) reuses its banks, so projections,
                # attention and o_proj can overlap on separate banks.
                with tc.tile_pool(name="sc", bufs=1, space="PSUM") as scp, \
                     tc.tile_pool(name="yt", bufs=1, space="PSUM") as ytp:
                    # ---- phase 1: projections + rope + transpose + V ----
                    if 1 in phases:
                        with tc.tile_pool(name="pp", bufs=1, space="PSUM") as pp:
                            ones_set = False
                            for m in range(SB):
                                ms = slice(m * 128, (m + 1) * 128)
                                # QK projection: [128 s, 512] = x_m @ [Wq|Wk]
                                ps = pp.tile([128, 2 * GDIM], F32, tag="ps_qk")
                                for k in range(KD):
                                    nc.tensor.matmul(ps[:], xt[:, k, ms], wqk[:, k, :],
                                                     start=(k == 0), stop=(k == KD - 1))
                                qkr = ropet.tile([128, 2 * GDIM], BF16, tag="qkr")
                                if use_rope:
                                    # single fast cast-copy releases the psum
                                    # slot; rope runs in bf16 on SBUF (2x DVE)
                                    qkf = ropet.tile([128, 2 * GDIM], BF16,
                                                     tag="qkf")
                                    nc.vector.tensor_copy(qkf[:], ps[:])
                                    pv = qkf.rearrange("p (x two) -> p two x", two=2)
                                    ov = qkr.rearrange("p (x two) -> p two x", two=2)
                                    E, O = pv[:, 0, :], pv[:, 1, :]
                                    C, Sn = cos8[:, m, :], sin8[:, m, :]
                                    ta = ropet.tile([128, 256], BF16, tag="ta")
                                    tb = ropet.tile([128, 256], BF16, tag="tb")
                                    nc.vector.tensor_mul(ta[:], E, C)
                                    nc.vector.tensor_mul(tb[:], O, Sn)
                                    nc.vector.tensor_sub(ov[:, 0, :], ta[:], tb[:])
                                    tc_ = ropet.tile([128, 256], BF16, tag="tc")
                                    td = ropet.tile([128, 256], BF16, tag="td")
                                    nc.vector.tensor_mul(tc_[:], O, C)
                                    nc.vector.tensor_mul(td[:], E, Sn)
                                    nc.vector.tensor_add(ov[:, 1, :], tc_[:], td[:])
                                else:
                                    nc.vector.tensor_copy(qkr[:], ps[:])
                                # transpose 128x128 blocks into qkt tiles
                                for cb in range(4):
                                    nc.sync.dma_start_transpose(
                                        qkt[cb][:, ms], qkr[:, cb * 128:(cb + 1) * 128])

                                # V projection: [128 s, 256]
                                psv = pp.tile([128, GDIM], F32, tag="ps_v")
                                for k in range(KD):
                                    nc.tensor.matmul(psv[:], xt[:, k, ms], wv[:, k, :],
                                                     start=(k == 0), stop=(k == KD - 1))
                                if not ones_set:
                                    nc.vector.memset(vsb[:], 1.0)
                                    ones_set = True
                                # copy 4 head blocks of 64 into stride-65 slots
                                dst = vsb[:, m, :].rearrange("p (h c) -> p h c", h=4)[:, :, 0:64]
                                src = psv.rearrange("p (h c) -> p h c", h=4)
                                nc.vector.tensor_copy(dst, src)

                    # ---- phase 2: attention, head pairs row-packed on PE ----
                    if 2 in phases:
                        # Heads 2p and 2p+1 share qkt tiles (partitions 0-63 / 64-127);
                        # their scoresT matmuls are issued to PE row groups 0 and 64 via
                        # tile_position auto-derivation and run concurrently.
                        for qc in range(NQC):
                            for hp in range(2):
                                qt = qkt[hp]
                                kt = qkt[2 + hp]
                                q0 = qc * QCHUNK
                                # Pack kb blocks into wide psum tiles of
                                # WIDE cols. A matmul may not cross a 512-col
                                # psum bank, so emit widths in order
                                # 512,...,512,384,128,256 (384+128=512 tiles
                                # banks exactly; 256 trails).
                                order = list(range(4 * qc)) + \
                                    [4 * qc, 4 * qc + 1, 4 * qc + 3, 4 * qc + 2]
                                groups, cur = [], []
                                cols = 0
                                for kb in order:
                                    r = max(0, kb - 4 * qc)
                                    qoff, n = q0 + r * 128, QCHUNK - r * 128
                                    if cols + n > WIDE:
                                        groups.append(cur)
                                        cur, cols = [], 0
                                    cur.append((kb, qoff, n, cols))
                                    cols += n
                                groups.append(cur)
                                last_kb = groups[-1][-1][0]


                                ytps = [ytp.tile([65, QCHUNK], F32,
                                                 tag=f"ytps{i}", name=f"ytps{i}")
                                        for i in range(2)]
                                for grp in groups:
                                    gcols = grp[-1][3] + grp[-1][2]
                                    scs = [scp.tile([128, WIDE], F32, tag=f"sc{i}",
                                                    name=f"sc{i}") for i in range(2)]
                                    for i in range(2):
                                        rows = slice(i * 64, i * 64 + 64)
                                        for (kb, qoff, n, o) in grp:
                                            nc.tensor.matmul(
                                                scs[i][:, o:o + n],
                                                kt[rows, kb * 128:(kb + 1) * 128],
                                                qt[rows, qoff:qoff + n],
                                                start=True, stop=True)
                                    for i in range(2):
                                        h = 2 * hp + i
                                        vcol = slice(h * 65, h * 65 + 65)
                                        pe = pex.tile([128, WIDE], BF16,
                                                      tag=f"pe{i}", name=f"pe{i}")
                                        nc.scalar.activation(pe[:, :gcols],
                                                             scs[i][:, :gcols],
                                                             EXP, scale=0.125)
                                        for (kb, qoff, n, o) in grp:
                                            if kb >= 4 * qc:  # diagonal: causal mask
                                                nc.vector.tensor_mul(
                                                    pe[:, o:o + 128], pe[:, o:o + 128],
                                                    maskT[:])
                                            # kb==0 always has n=512: start clears
                                            # the whole [65, QCHUNK] accumulator
                                            nc.tensor.matmul(
                                                ytps[i][:, qoff - q0:qoff - q0 + n],
                                                vsb[:, kb, vcol],
                                                pe[:, o:o + n],
                                                start=(kb == 0), stop=(kb == last_kb))
                                for i in range(2):
                                    h = 2 * hp + i
                                    # single copy releases the psum bank for
                                    # the next chunk's PV; normalize from SBUF
                                    ytu = work.tile([65, QCHUNK], F32, tag="ytu")
                                    nc.vector.tensor_copy(ytu[:], ytps[i][:])
                                    rc = work.tile([1, QCHUNK], F32, tag="rc")
                                    nc.vector.reciprocal(rc[:], ytu[64:65, :])
                                    bc = work.tile([64, QCHUNK], F32, tag="bc")
                                    nc.gpsimd.partition_broadcast(bc[:], rc[0:1, :])
                                    nc.vector.tensor_mul(
                                        yt2[hp][i * 64:i * 64 + 64, q0:q0 + QCHUNK],
                                        ytu[0:64, :], bc[:])

                    # ---- phase 3: o_proj ----
                    if 3 in phases:
                        with tc.tile_pool(name="op", bufs=2, space="PSUM") as op:
                            for m in range(SB):
                                ms = slice(m * 128, (m + 1) * 128)
                                for nb in range(2):
                                    po = op.tile([128, 512], F32, tag="po")
                                    for k in range(2):
                                        nc.tensor.matmul(po[:], yt2[k][:, ms],
                                                         wo[:, k, nb * 512:(nb + 1) * 512],
                                                         start=(k == 0), stop=(k == 1))
                                    so = work.tile([128, 512], F32, tag="so")
                                    nc.vector.tensor_copy(so[:], po[:])
                                    nc.sync.dma_start(
                                        out_d[ms, nb * 512:(nb + 1) * 512], so[:])
                                    if timing and out_small is not None and m == 0 and nb == 0:
                                        nc.sync.dma_start(out_small[:], so[:])
    nc.compile()
    return nc


def _prep_core_inputs(x, Wq, Wk, Wv, Wo, cos_g, sin_g, use_rope):
    """Host-side shard + layout prep. Returns list of 8 input dicts."""
    maskT = np.tril(np.ones((128, 128), np.float32)).T.astype(_BF16)
    # interleave cos/sin to the 256-wide repeating pattern used by rope
    cos8 = np.tile(cos_g, (1, 8)).astype(_BF16)
    sin8 = np.tile(sin_g, (1, 8)).astype(_BF16)
    maps = []
    for c in range(NCORES):
        b, g = divmod(c, HEADS_PER_CORE)
        rows = slice(g * GDIM, (g + 1) * GDIM)
        wqk = np.concatenate([Wq[rows], Wk[rows]], axis=0).T  # [D, 512]
        maps.append({
            "xt": np.ascontiguousarray(x[b].T).astype(_BF16),
            "wqk": np.ascontiguousarray(wqk).astype(_BF16),
            "wv": np.ascontiguousarray(Wv[rows].T).astype(_BF16),
            "wo": np.ascontiguousarray(Wo[:, rows].T).astype(_BF16),
            "cos8": cos8,
            "sin8": sin8,
            "maskT": maskT,
        })
    return maps


def kernel(x, token_positions, use_rope, Wq, Wk, Wv, Wo, cos, sin):
    from concourse.bass_utils import run_bass_kernel_spmd

    x = np.asarray(x, np.float32)
    token_positions = np.asarray(token_positions)
    Wq = np.asarray(Wq, np.float32)
    Wk = np.asarray(Wk, np.float32)
    Wv = np.asarray(Wv, np.float32)
    Wo = np.asarray(Wo, np.float32)
    cos = np.asarray(cos, np.float32)
    sin = np.asarray(sin, np.float32)
    rope = bool(int(use_rope))

    cos_g = cos[token_positions]  # [S, 32]
    sin_g = sin[token_positions]

    if rope not in _cache:
        _cache[rope] = _build(rope)
    nc = _cache[rope]

    in_maps = _prep_core_inputs(x, Wq, Wk, Wv, Wo, cos_g, sin_g, rope)
    res = run_bass_kernel_spmd(nc, in_maps, list(range(NCORES)))

    out = np.zeros((B, S, D), np.float32)
    for c in range(NCORES):
        out[c // HEADS_PER_CORE] += res.results[c]["out"]
    return out



# revision 105
# speedup vs baseline: 1.3753x; 1.3753x over previous
"""Causal multi-head attention with RoPE for Trainium2, 8-core SPMD.

Problem: B=2, S=2048, D_MODEL=1024, H=16, HD=64, causal softmax(QK^T/8)V
with interleaved-pair RoPE on q/k, projections Wq/Wk/Wv/Wo.

Sharding (host side): batch x head-group. Core c handles batch b=c//4 and
head group g=c%4 (heads 4g..4g+3, a 256-wide slice of the projection dims).
Each core computes a full [S, D_MODEL] partial of the output (its head
group's contribution through Wo); host sums 4 partials per batch.

Device strategy (cost-model-driven):
 - QKV projections run in fp8e4m3 with DoubleRow perf mode (one matmul
   contracts 2x128 of D at 0.5 cycles/col). Host scales W by 32 so fp8
   never hits denormals; the 32x factors cancel in softmax and are folded
   into Wo.
 - RoPE: host permutes Wq/Wk output cols within each head to [even|odd]
   blocks, so rope is 2 DVE muls (cos/sin tables, stride-0 broadcast per
   head) + 2 GPSIMD combines on contiguous 32-col slices.
 - Q/K transposed to [o, s] via one XBAR DMA per m-tile (3D out AP).
 - scores^T[k, q]: per q-block qb, kb key-blocks for a PAIR of heads land
   in wide [128, 1024] PSUM tiles so Exp on ACT amortizes its init cost.
   Causal: only kb<=qb computed; diagonal masked by gpsimd multiplies.
 - PV is flipped: lhsT = pe (exp'd scores, [keys, q]) so the output is
   [q, hd] with q on PSUM partitions; rhs = [V | 1] so column 64 of each
   head's slice accumulates the softmax denominator. Normalization is a
   [128,4] reciprocal + one stride-0-broadcast multiply per q-block.
 - y [q, 256] -> XBAR -> yT for o_proj (bf16), partial out stored bf16;
   host sums 4 partials per batch in f32.
"""

import numpy as np
import ml_dtypes

B, S, D, H = 2, 2048, 1024, 16
HD = 64
NCORES = 8
HEADS_PER_CORE = 4
GDIM = HEADS_PER_CORE * HD          # 256 projection cols per core
SB = S // 128                        # 16 s-tiles / q-blocks
WSCALE = 32.0
CHUNK = 4                            # kb blocks per scores chunk (x2 heads)

_BF16 = ml_dtypes.bfloat16
_FP8 = ml_dtypes.float8_e4m3
_cache = {}


def _build(use_rope: bool):
    import concourse.bass as bass
    import concourse.mybir as mybir
    import concourse.tile as tile
    from concourse import bacc

    F32 = mybir.dt.float32
    BF16 = mybir.dt.bfloat16
    FP8 = mybir.dt.float8e4
    EXP = mybir.ActivationFunctionType.Exp
    DR = mybir.MatmulPerfMode.DoubleRow

    nc = bacc.Bacc(None, target_bir_lowering=False)

    # host pre-tiles xt8 to [p, m, k, s'] and rope tables to [p, cc|ss, m, f]
    # so every load DMA moves >=1KB-contiguous rows per partition.
    # fp8 residual correction: k planes 0-7 = fp8(x), planes 8-15 =
    # fp8(x - fp8(x)); w8 cols 0-767 = fp8(32W), 768-1535 = fp8(32W - fp8(32W)).
    # Projections accumulate X8@W8 + X8@E8 + R8@W8 (all DoubleRow).
    xt_d = nc.dram_tensor("xt8", [128, SB, 16, 128], FP8, kind="ExternalInput")
    w8_d = nc.dram_tensor("w8", [D, 1536], FP8, kind="ExternalInput")
    wo_d = nc.dram_tensor("wo", [GDIM, D], BF16, kind="ExternalInput")
    rope_d = nc.dram_tensor("ropetab", [128, 2, SB, 64], BF16,
                            kind="ExternalInput")
    mask_d = nc.dram_tensor("maskT", [128, 128], BF16, kind="ExternalInput")
    ident_d = nc.dram_tensor("ident", [128, 128], BF16, kind="ExternalInput")
    out_d = nc.dram_tensor("out", [S, D], BF16, kind="ExternalOutput")

    ESCALE = 0.125 / (WSCALE * WSCALE)

    with tile.TileContext(nc) as tc:
        with tc.tile_pool(name="big", bufs=1) as big, \
             tc.tile_pool(name="work", bufs=3) as work, \
             tc.tile_pool(name="pex", bufs=9) as pex, \
             tc.tile_pool(name="scA", bufs=3, space="PSUM") as scA, \
             tc.tile_pool(name="yp", bufs=1, space="PSUM") as ypp, \
             tc.tile_pool(name="shp", bufs=1, space="PSUM") as shp:
            # ---- resident tensors ----
            # Load order unblocks proj(0) fast: w8, x chunk 0, rope tables.
            # w8 split into 4 tiles so the first QK matmuls unblock early
            w8src = w8_d.rearrange("(k p) o -> p k o", p=128)
            w8qk = big.tile([128, 8, 512], FP8)
            nc.sync.dma_start(w8qk[:], w8src[:, :, 0:512])
            # xt8 split into chunks (separate tiles -> no false deps): each
            # chunk covers an m-range; proj(m) reads exactly one chunk.
            xchunks = [(0, 2), (2, 4), (4, 8), (8, 16)]
            xt8s = {}
            xtiles = {}

            def load_xchunk(ci):
                m0, m1 = xchunks[ci]
                t = big.tile([128, m1 - m0, 16, 128], FP8,
                             tag=f"xt{m0}", name=f"xt{m0}")
                nc.sync.dma_start(t[:], xt_d[:, m0:m1, :, :])
                for m in range(m0, m1):
                    xt8s[m] = (t, m - m0)

            # first x chunk + rope tables ride the ACT HWDGE queue so their
            # DGE phases overlap the SP-queue weight loads
            m0, m1 = xchunks[0]
            t0 = big.tile([128, m1 - m0, 16, 128], FP8, tag="xt0", name="xt0")
            nc.scalar.dma_start(t0[:], xt_d[:, m0:m1, :, :])
            for m in range(m0, m1):
                xt8s[m] = (t0, m - m0)
            if use_rope:
                ropet = big.tile([128, 2, SB, 64], BF16)
                nc.scalar.dma_start(ropet[:], rope_d[:])
            e8qk = big.tile([128, 8, 512], FP8)
            nc.sync.dma_start(e8qk[:], w8src[:, :, 768:1280])
            w8v = big.tile([128, 8, 256], FP8)
            nc.sync.dma_start(w8v[:], w8src[:, :, 512:768])
            load_xchunk(1)
            e8v = big.tile([128, 8, 256], FP8)
            nc.sync.dma_start(e8v[:], w8src[:, :, 1280:1536])
            maskT = big.tile([128, 128], BF16)
            nc.sync.dma_start(maskT[:], mask_d[:])
            ident = big.tile([128, 128], BF16)
            nc.sync.dma_start(ident[:], ident_d[:])
            load_xchunk(2)
            wo = big.tile([128, 2, D], BF16)
            nc.sync.dma_start(wo[:], wo_d.rearrange("(k p) o -> p k o", p=128))
            load_xchunk(3)

            vsb = big.tile([128, SB, HEADS_PER_CORE * 65], BF16)
            # only the per-head "ones" columns (col 64 of each 65-block)
            nc.vector.memset(
                vsb.rearrange("p m (h c) -> p m h c", c=65)[:, :, :, 64], 1.0)
            # per-m Q/K transposed tiles; j: 0 = Q h0,h1 ; 1 = Q h2,h3 ;
            # 2 = K h0,h1 ; 3 = K h2,h3
            qkt = [big.tile([128, 4, 128], BF16, tag=f"qkt{m}", name=f"qkt{m}")
                   for m in range(SB)]
            yt = [big.tile([128, 2, 128], BF16, tag=f"yt{m}", name=f"yt{m}")
                  for m in range(SB)]

            # ---------------- emission helpers ----------------
            proj_ps = {}
            projv_ps = {}

            def proj_qk_term(m, ti):
                xt_t, moff = xt8s[m]
                if ti == 0:
                    # early tiles ride the (then-idle) scores slots so the
                    # prologue projection chains pipeline 3-deep
                    if m < 2:
                        ps = scA.tile([128, 1024], F32, tag="scA",
                                      name=f"ppqk{m}")[:, 0:512]
                    else:
                        ps = shp.tile([128, 512], F32, tag="shp",
                                      name=f"ppqk{m}")
                    proj_ps[m] = ps
                else:
                    ps = proj_ps[m]
                xo, wt = ((0, w8qk), (0, e8qk), (8, w8qk))[ti]
                for k in range(4):
                    nc.tensor.matmul(
                        ps[:], xt_t[:, moff, xo + 2 * k:xo + 2 * k + 2, :],
                        wt[:, 2 * k:2 * k + 2, :],
                        start=(ti == 0 and k == 0),
                        stop=(ti == 2 and k == 3), perf_mode=DR)

            def proj_qk_tail(m):
                ps = proj_ps.pop(m)
                qkf = work.tile([128, 512], BF16, tag="qkf")
                nc.vector.tensor_copy(qkf[:], ps[:])
                if use_rope:
                    ccv = ropet[:, 0, m, :].unsqueeze(1).broadcast_to([128, 8, 64])
                    ssv = ropet[:, 1, m, :].unsqueeze(1).broadcast_to([128, 8, 64])
                    ta = work.tile([128, 512], BF16, tag="ta")
                    tav = ta.rearrange("p (h t e) -> p h t e", t=2, e=32)
                    tb = work.tile([128, 512], BF16, tag="tb")
                    tbv = tb.rearrange("p (h t e) -> p h t e", t=2, e=32)
                    nc.vector.tensor_mul(
                        ta.rearrange("p (h f) -> p h f", f=64),
                        qkf.rearrange("p (h f) -> p h f", f=64), ccv)
                    nc.vector.tensor_mul(
                        tb.rearrange("p (h f) -> p h f", f=64),
                        qkf.rearrange("p (h f) -> p h f", f=64), ssv)
                    qkr = work.tile([128, 512], BF16, tag="qkr")
                    qrv = qkr.rearrange("p (h t e) -> p h t e", t=2, e=32)
                    # outE = E*c - O*s ; outO = O*c + E*s
                    nc.gpsimd.tensor_sub(qrv[:, :, 0, :], tav[:, :, 0, :],
                                         tbv[:, :, 1, :])
                    nc.gpsimd.tensor_add(qrv[:, :, 1, :], tav[:, :, 1, :],
                                         tbv[:, :, 0, :])
                else:
                    qkr = qkf
                nc.sync.dma_start_transpose(qkt[m][:], qkr[:])

            def proj_v_term(m, ti):
                xt_t, moff = xt8s[m]
                if ti == 0:
                    if m < 2:
                        ps = scA.tile([128, 1024], F32, tag="scA",
                                      name=f"ppv{m}")[:, 0:256]
                    elif m < 6:
                        # early: the ypsum bank is mostly idle; keeps the V
                        # chain out of the shp slot so QK/V pipeline
                        ps = ypp.tile([128, HEADS_PER_CORE * 65], F32,
                                      tag="yp", name=f"ppv{m}")[:, 0:256]
                    else:
                        ps = shp.tile([128, 512], F32, tag="shp",
                                      name=f"ppv{m}")[:, 0:256]
                    projv_ps[m] = ps
                else:
                    ps = projv_ps[m]
                xo, wt = ((0, w8v), (0, e8v), (8, w8v))[ti]
                for k in range(4):
                    nc.tensor.matmul(
                        ps[:], xt_t[:, moff, xo + 2 * k:xo + 2 * k + 2, :],
                        wt[:, 2 * k:2 * k + 2, :],
                        start=(ti == 0 and k == 0),
                        stop=(ti == 2 and k == 3), perf_mode=DR)
                if ti == 2:
                    ps = projv_ps.pop(m)
                    dst = vsb[:, m, :].rearrange(
                        "p (h c) -> p h c", h=4)[:, :, 0:64]
                    src = ps.rearrange("p (h c) -> p h c", h=4)
                    nc.vector.tensor_copy(dst, src)

            def oproj_nb(m, nb, so):
                if m >= 13:  # attention is winding down; scores slots idle
                    po = scA.tile([128, 1024], F32, tag="scA",
                                  name=f"po{m}_{nb}")[:, 0:512]
                else:
                    po = shp.tile([128, 512], F32, tag="shp",
                                  name=f"po{m}_{nb}")
                for k in range(2):
                    nc.tensor.matmul(po[:], yt[m][:, k, :],
                                     wo[:, k, nb * 512:(nb + 1) * 512],
                                     start=(k == 0), stop=(k == 1))
                # high priority: the copy releases the single op psum bank,
                # which gates the next o_proj piece's matmuls on PE
                with tc.high_priority(offset=1000):
                    nc.vector.tensor_copy(so[:, nb * 512:(nb + 1) * 512], po[:])

            def proj_qk(m):
                proj_qk_term(m, 0)
                proj_qk_term(m, 1)
                proj_qk_term(m, 2)
                proj_qk_tail(m)

            def proj_v(m):
                proj_v_term(m, 0)
                proj_v_term(m, 1)
                proj_v_term(m, 2)

            # -------------- prologue: projections m=0,1 --------------
            # remaining projections staged across early (attention-light)
            # iterations: two per qb while qb<5, then one per qb
            proj_sched = {qb: [] for qb in range(SB)}
            for m in range(2):
                proj_qk(m)
                proj_v(m)
            for qb in range(5):
                proj_sched[qb] = [2 + 2 * qb, 3 + 2 * qb]
            for qb in range(5, 9):
                proj_sched[qb] = [12 + (qb - 5)]

            # o_proj work queue: (m, nb) pieces plus stores, consumed at a
            # lag of ~4 q-blocks so yt[m] transposes are long since done.
            oproj_work = []

            def oproj_piece():
                if not oproj_work:
                    return
                fn = oproj_work.pop(0)
                fn()

            so_tiles = {}
            opush_next = 0

            def push_oproj(m):
                def nb0():
                    so_tiles[m] = work.tile([128, D], BF16, tag="so",
                                            name=f"so{m}")
                    oproj_nb(m, 0, so_tiles[m])
                    if m >= 14:  # tail: don't gate the store on both halves
                        nc.sync.dma_start(out_d[m * 128:(m + 1) * 128, 0:512],
                                          so_tiles[m][:, 0:512])

                def nb1():
                    so = so_tiles.pop(m)
                    oproj_nb(m, 1, so)
                    if m >= 14:
                        nc.sync.dma_start(out_d[m * 128:(m + 1) * 128, 512:D],
                                          so[:, 512:D])
                    else:
                        nc.sync.dma_start(out_d[m * 128:(m + 1) * 128, :], so[:])

                oproj_work.append(nb0)
                oproj_work.append(nb1)

            # -------------- attention machinery (cross-qb pipelined) -------
            def chunks_of(qb):
                return [list(range(c0, min(c0 + CHUNK, qb + 1)))
                        for c0 in range(0, qb + 1, CHUNK)]

            pes = {}
            ypsums = {}
            sc_done = set()

            def scores(qb, hp, ci):
                # hi=0 blocks live in bank 0 (cols 0:512), hi=1 in bank 1
                # (cols 512:1024): matmuls with different PE row-group
                # tile_positions must not share a PSUM bank.
                if (qb, hp, ci) in sc_done:
                    return
                sc_done.add((qb, hp, ci))
                kbs = chunks_of(qb)[ci]
                w = len(kbs)
                sc = scA.tile([128, 1024], F32, tag="scA",
                              name=f"sc{qb}_{hp}_{ci}")
                for hi in range(2):
                    rows = slice(hi * 64, hi * 64 + 64)
                    for j, kb in enumerate(kbs):
                        nc.tensor.matmul(
                            sc[:, (hi * 4 + j) * 128:(hi * 4 + j + 1) * 128],
                            qkt[kb][rows, 2 + hp, :],
                            qkt[qb][rows, hp, :],
                            start=True, stop=True)
                pe_t = pex.tile([128, 1024], BF16, tag="pe",
                                name=f"pe{qb}_{hp}_{ci}")
                if w <= 2:  # two narrow exps beat spanning the bank gap
                    nc.scalar.activation(pe_t[:, 0:w * 128],
                                         sc[:, 0:w * 128], EXP, scale=ESCALE)
                    nc.scalar.activation(pe_t[:, 512:(4 + w) * 128],
                                         sc[:, 512:(4 + w) * 128], EXP,
                                         scale=ESCALE)
                else:
                    nc.scalar.activation(pe_t[:, 0:(4 + w) * 128],
                                         sc[:, 0:(4 + w) * 128], EXP,
                                         scale=ESCALE)
                if kbs[-1] == qb:  # causal diagonal block
                    for hi in range(2):
                        off = (hi * 4 + w - 1) * 128
                        nc.gpsimd.tensor_mul(
                            pe_t[:, off:off + 128],
                            pe_t[:, off:off + 128], maskT[:])
                pes[(qb, hp, ci)] = (pe_t, kbs)

            def pv_full(qb, hp):
                # One head's ENTIRE accumulation chain is contiguous: a PSUM
                # bank supports only one open accumulation group, so
                # interleaving two heads' start..stop chains in the same bank
                # corrupts the earlier one.
                if qb not in ypsums:
                    ypsums[qb] = ypp.tile([128, HEADS_PER_CORE * 65], F32,
                                          tag="yp", name=f"yp{qb}")
                ypsum = ypsums[qb]
                nch = len(chunks_of(qb))
                for hi in range(2):
                    h = 2 * hp + hi
                    vcol = slice(h * 65, h * 65 + 65)
                    for ci in range(nch):
                        pe_t, kbs = pes[(qb, hp, ci)]
                        for j, kb in enumerate(kbs):
                            nc.tensor.matmul(
                                ypsum[:, vcol],
                                pe_t[:, (hi * 4 + j) * 128:(hi * 4 + j + 1) * 128],
                                vsb[:, kb, vcol],
                                start=(kb == 0), stop=(kb == qb),
                                skip_group_check=True)
                for ci in range(nch):
                    pes.pop((qb, hp, ci))

            def normalize(qb):
                # high priority: frees the single ypsum bank for qb+1's PV
                ypsum = ypsums.pop(qb)
                yview = ypsum.rearrange("p (h c) -> p h c", c=65)
                with tc.high_priority(offset=1000):
                    rc = work.tile([128, 4], F32, tag="rc", name=f"rc{qb}")
                    nc.vector.reciprocal(rc[:], yview[:, :, 64])
                    ynorm = work.tile([128, 4, 64], BF16, tag="ynorm",
                                      name=f"yn{qb}")
                    nc.vector.tensor_mul(
                        ynorm[:], yview[:, :, 0:64],
                        rc.unsqueeze(2).broadcast_to([128, 4, 64]))
                if qb >= 14:
                    # tail: PE transpose (via idle scores slot) beats the
                    # ~2.4us XBAR DMA latency on the critical path
                    tp = scA.tile([128, 1024], F32, tag="scA",
                                  name=f"ytp{qb}").bitcast(BF16)
                    ynf = ynorm.rearrange("p h e -> p (h e)")
                    with tc.high_priority(offset=1000):
                        for b in range(2):
                            nc.tensor.transpose(
                                tp[:, b * 128:(b + 1) * 128],
                                ynf[:, b * 128:(b + 1) * 128], ident[:])
                        nc.vector.tensor_copy(
                            yt[qb][:], tp[:, 0:256].rearrange(
                                "p (a b) -> p a b", a=2))
                else:
                    nc.sync.dma_start_transpose(
                        yt[qb][:], ynorm.rearrange("p h e -> p (h e)"))

            # -------------- main loop over q-blocks --------------
            for qb in range(SB):
                nchunks = len(chunks_of(qb))
                # push o_proj work: lag 4 normally; from qb=13 push two per
                # iteration (lag >=1 on yt is safe) to drain the queue early
                target = qb - 4 if qb <= 12 else min(8 + 2 * (qb - 12), SB - 2)
                while opush_next <= target:
                    push_oproj(opush_next)
                    opush_next += 1
                npop = 2 if qb < 10 else 3

                pieces = []
                for m_ in proj_sched[qb]:
                    pieces.append(lambda m=m_: (proj_qk_term(m, 0),
                                                proj_qk_term(m, 1)))
                    pieces.append(lambda m=m_: (proj_qk_term(m, 2),
                                                proj_qk_tail(m)))
                    pieces.append(lambda m=m_: (proj_v_term(m, 0),
                                                proj_v_term(m, 1)))
                    pieces.append(lambda m=m_: proj_v_term(m, 2))
                for _ in range(npop):
                    pieces.append(oproj_piece)

                def filler(pieces=pieces):
                    if pieces:
                        pieces.pop(0)()

                for ci in range(nchunks):
                    scores(qb, 0, ci)   # may be a prefetched no-op
                    filler()
                scores(qb, 1, 0)
                filler()
                pv_full(qb, 0)
                for ci in range(1, nchunks):
                    scores(qb, 1, ci)
                    filler()
                while pieces:
                    filler()
                # prefetch next qb's first scores units to cover the
                # pv_full(1) + normalize latency at the iteration boundary
                if qb + 1 < SB:
                    scores(qb + 1, 0, 0)
                    if len(chunks_of(qb + 1)) > 1:
                        scores(qb + 1, 0, 1)
                pv_full(qb, 1)
                normalize(qb)

            # -------------- epilogue: drain remaining o_proj --------------
            push_oproj(SB - 1)
            while oproj_work:
                oproj_piece()

    nc.compile()
    return nc


def _perm_eo():
    """Per-head column permutation: [0,2,...,62, 1,3,...,63]."""
    return np.concatenate([np.arange(0, HD, 2), np.arange(1, HD, 2)])


def _prep_core_inputs(x, Wq, Wk, Wv, Wo, cos_g, sin_g, use_rope):
    maskT = np.tril(np.ones((128, 128), np.float32)).T.astype(_BF16)
    # rope tables tiled to [p, cc|ss, m, f] (f pattern = [cos|cos]/[sin|sin])
    cc = np.concatenate([cos_g, cos_g], axis=1)                  # [S, 64]
    ss = np.concatenate([sin_g, sin_g], axis=1)
    ropetab = np.stack([
        cc.reshape(SB, 128, 64).transpose(1, 0, 2),
        ss.reshape(SB, 128, 64).transpose(1, 0, 2)], axis=1).astype(_BF16)
    perm = _perm_eo()
    maps = []
    for c in range(NCORES):
        b, g = divmod(c, HEADS_PER_CORE)
        rows = slice(g * GDIM, (g + 1) * GDIM)
        wq = Wq[rows].reshape(HEADS_PER_CORE, HD, D)[:, perm, :].reshape(GDIM, D)
        wk = Wk[rows].reshape(HEADS_PER_CORE, HD, D)[:, perm, :].reshape(GDIM, D)
        w32 = np.concatenate([wq, wk, Wv[rows]], axis=0).T * WSCALE  # [D, 768]
        w8 = w32.astype(_FP8)
        e8 = (w32 - w8.astype(np.float32)).astype(_FP8)
        w8full = np.concatenate([w8, e8], axis=1)                   # [D, 1536]
        # xt8: [p, m, k, s'] with element (p,m,k,s') = x[b][m*128+s', k*128+p];
        # k planes 8-15 hold the fp8 residual of x
        xtl = x[b].T.reshape(8, 128, SB, 128).transpose(1, 2, 0, 3)
        x8 = xtl.astype(_FP8)
        r8 = (xtl - x8.astype(np.float32)).astype(_FP8)
        xt8 = np.concatenate([x8, r8], axis=2)       # [128, SB, 16, 128]
        maps.append({
            "xt8": np.ascontiguousarray(xt8),
            "w8": np.ascontiguousarray(w8full),
            "wo": np.ascontiguousarray(Wo[:, rows].T / WSCALE).astype(_BF16),
            "ropetab": ropetab,
            "maskT": maskT,
            "ident": np.eye(128, dtype=np.float32).astype(_BF16),
        })
    return maps


def kernel(x, token_positions, use_rope, Wq, Wk, Wv, Wo, cos, sin):
    from concourse.bass_utils import run_bass_kernel_spmd

    x = np.asarray(x, np.float32)
    token_positions = np.asarray(token_positions)
    Wq = np.asarray(Wq, np.float32)
    Wk = np.asarray(Wk, np.float32)
    Wv = np.asarray(Wv, np.float32)
    Wo = np.asarray(Wo, np.float32)
    cos = np.asarray(cos, np.float32)
    sin = np.asarray(sin, np.float32)
    rope = bool(int(use_rope))

    cos_g = cos[token_positions]  # [S, 32]
    sin_g = sin[token_positions]

    if rope not in _cache:
        _cache[rope] = _build(rope)
    nc = _cache[rope]

    in_maps = _prep_core_inputs(x, Wq, Wk, Wv, Wo, cos_g, sin_g, rope)
    res = run_bass_kernel_spmd(nc, in_maps, list(range(NCORES)))

    out = np.zeros((B, S, D), np.float32)
    for c in range(NCORES):
        out[c // HEADS_PER_CORE] += res.results[c]["out"].astype(np.float32)
    return out


# revision 106
# speedup vs baseline: 1.3784x; 1.0023x over previous
"""Causal multi-head attention with RoPE for Trainium2, 8-core SPMD.

Problem: B=2, S=2048, D_MODEL=1024, H=16, HD=64, causal softmax(QK^T/8)V
with interleaved-pair RoPE on q/k, projections Wq/Wk/Wv/Wo.

Sharding (host side): batch x head-group. Core c handles batch b=c//4 and
head group g=c%4 (heads 4g..4g+3, a 256-wide slice of the projection dims).
Each core computes a full [S, D_MODEL] partial of the output (its head
group's contribution through Wo); host sums 4 partials per batch.

Device strategy (cost-model-driven):
 - QKV projections run in fp8e4m3 with DoubleRow perf mode (one matmul
   contracts 2x128 of D at 0.5 cycles/col). Host scales W by 32 so fp8
   never hits denormals; the 32x factors cancel in softmax and are folded
   into Wo.
 - RoPE: host permutes Wq/Wk output cols within each head to [even|odd]
   blocks, so rope is 2 DVE muls (cos/sin tables, stride-0 broadcast per
   head) + 2 GPSIMD combines on contiguous 32-col slices.
 - Q/K transposed to [o, s] via one XBAR DMA per m-tile (3D out AP).
 - scores^T[k, q]: per q-block qb, kb key-blocks for a PAIR of heads land
   in wide [128, 1024] PSUM tiles so Exp on ACT amortizes its init cost.
   Causal: only kb<=qb computed; diagonal masked by gpsimd multiplies.
 - PV is flipped: lhsT = pe (exp'd scores, [keys, q]) so the output is
   [q, hd] with q on PSUM partitions; rhs = [V | 1] so column 64 of each
   head's slice accumulates the softmax denominator. Normalization is a
   [128,4] reciprocal + one stride-0-broadcast multiply per q-block.
 - y [q, 256] -> XBAR -> yT for o_proj (bf16), partial out stored bf16;
   host sums 4 partials per batch in f32.
"""

import numpy as np
import ml_dtypes

B, S, D, H = 2, 2048, 1024, 16
HD = 64
NCORES = 8
HEADS_PER_CORE = 4
GDIM = HEADS_PER_CORE * HD          # 256 projection cols per core
SB = S // 128                        # 16 s-tiles / q-blocks
WSCALE = 32.0
CHUNK = 4                            # kb blocks per scores chunk (x2 heads)

_BF16 = ml_dtypes.bfloat16
_FP8 = ml_dtypes.float8_e4m3
_cache = {}


def _build(use_rope: bool):
    import concourse.bass as bass
    import concourse.mybir as mybir
    import concourse.tile as tile
    from concourse import bacc

    F32 = mybir.dt.float32
    BF16 = mybir.dt.bfloat16
    FP8 = mybir.dt.float8e4
    EXP = mybir.ActivationFunctionType.Exp
    DR = mybir.MatmulPerfMode.DoubleRow

    nc = bacc.Bacc(None, target_bir_lowering=False)

    # host pre-tiles xt8 to [p, m, k, s'] and rope tables to [p, cc|ss, m, f]
    # so every load DMA moves >=1KB-contiguous rows per partition.
    # fp8 residual correction: k planes 0-7 = fp8(x), planes 8-15 =
    # fp8(x - fp8(x)); w8 cols 0-767 = fp8(32W), 768-1535 = fp8(32W - fp8(32W)).
    # Projections accumulate X8@W8 + X8@E8 + R8@W8 (all DoubleRow).
    xt_d = nc.dram_tensor("xt8", [128, SB, 16, 128], FP8, kind="ExternalInput")
    w8_d = nc.dram_tensor("w8", [D, 1536], FP8, kind="ExternalInput")
    wo_d = nc.dram_tensor("wo", [GDIM, D], BF16, kind="ExternalInput")
    rope_d = nc.dram_tensor("ropetab", [128, 2, SB, 64], BF16,
                            kind="ExternalInput")
    mask_d = nc.dram_tensor("maskT", [128, 128], BF16, kind="ExternalInput")
    ident_d = nc.dram_tensor("ident", [128, 128], BF16, kind="ExternalInput")
    out_d = nc.dram_tensor("out", [S, D], BF16, kind="ExternalOutput")

    ESCALE = 0.125 / (WSCALE * WSCALE)

    with tile.TileContext(nc) as tc:
        with tc.tile_pool(name="big", bufs=1) as big, \
             tc.tile_pool(name="work", bufs=4) as work, \
             tc.tile_pool(name="pex", bufs=11) as pex, \
             tc.tile_pool(name="scA", bufs=3, space="PSUM") as scA, \
             tc.tile_pool(name="yp", bufs=1, space="PSUM") as ypp, \
             tc.tile_pool(name="shp", bufs=1, space="PSUM") as shp:
            # ---- resident tensors ----
            # Load order unblocks proj(0) fast: w8, x chunk 0, rope tables.
            # w8 split into 4 tiles so the first QK matmuls unblock early
            w8src = w8_d.rearrange("(k p) o -> p k o", p=128)
            w8qk = big.tile([128, 8, 512], FP8)
            nc.sync.dma_start(w8qk[:], w8src[:, :, 0:512])
            # xt8 split into chunks (separate tiles -> no false deps): each
            # chunk covers an m-range; proj(m) reads exactly one chunk.
            xchunks = [(0, 2), (2, 4), (4, 8), (8, 16)]
            xt8s = {}
            xtiles = {}

            def load_xchunk(ci):
                m0, m1 = xchunks[ci]
                t = big.tile([128, m1 - m0, 16, 128], FP8,
                             tag=f"xt{m0}", name=f"xt{m0}")
                nc.sync.dma_start(t[:], xt_d[:, m0:m1, :, :])
                for m in range(m0, m1):
                    xt8s[m] = (t, m - m0)

            # first x chunk + rope tables ride the ACT HWDGE queue so their
            # DGE phases overlap the SP-queue weight loads
            m0, m1 = xchunks[0]
            t0 = big.tile([128, m1 - m0, 16, 128], FP8, tag="xt0", name="xt0")
            nc.scalar.dma_start(t0[:], xt_d[:, m0:m1, :, :])
            for m in range(m0, m1):
                xt8s[m] = (t0, m - m0)
            if use_rope:
                ropet = big.tile([128, 2, SB, 64], BF16)
                nc.scalar.dma_start(ropet[:], rope_d[:])
            e8qk = big.tile([128, 8, 512], FP8)
            nc.sync.dma_start(e8qk[:], w8src[:, :, 768:1280])
            w8v = big.tile([128, 8, 256], FP8)
            nc.sync.dma_start(w8v[:], w8src[:, :, 512:768])
            load_xchunk(1)
            e8v = big.tile([128, 8, 256], FP8)
            nc.sync.dma_start(e8v[:], w8src[:, :, 1280:1536])
            maskT = big.tile([128, 128], BF16)
            nc.sync.dma_start(maskT[:], mask_d[:])
            ident = big.tile([128, 128], BF16)
            nc.sync.dma_start(ident[:], ident_d[:])
            load_xchunk(2)
            wo = big.tile([128, 2, D], BF16)
            nc.sync.dma_start(wo[:], wo_d.rearrange("(k p) o -> p k o", p=128))
            load_xchunk(3)

            vsb = big.tile([128, SB, HEADS_PER_CORE * 65], BF16)
            # only the per-head "ones" columns (col 64 of each 65-block)
            nc.vector.memset(
                vsb.rearrange("p m (h c) -> p m h c", c=65)[:, :, :, 64], 1.0)
            # per-m Q/K transposed tiles; j: 0 = Q h0,h1 ; 1 = Q h2,h3 ;
            # 2 = K h0,h1 ; 3 = K h2,h3
            qkt = [big.tile([128, 4, 128], BF16, tag=f"qkt{m}", name=f"qkt{m}")
                   for m in range(SB)]
            yt = [big.tile([128, 2, 128], BF16, tag=f"yt{m}", name=f"yt{m}")
                  for m in range(SB)]

            # ---------------- emission helpers ----------------
            proj_ps = {}
            projv_ps = {}

            def proj_qk_term(m, ti):
                xt_t, moff = xt8s[m]
                if ti == 0:
                    # early tiles ride the (then-idle) scores slots so the
                    # prologue projection chains pipeline 3-deep
                    if m < 2:
                        ps = scA.tile([128, 1024], F32, tag="scA",
                                      name=f"ppqk{m}")[:, 0:512]
                    else:
                        ps = shp.tile([128, 512], F32, tag="shp",
                                      name=f"ppqk{m}")
                    proj_ps[m] = ps
                else:
                    ps = proj_ps[m]
                xo, wt = ((0, w8qk), (0, e8qk), (8, w8qk))[ti]
                for k in range(4):
                    nc.tensor.matmul(
                        ps[:], xt_t[:, moff, xo + 2 * k:xo + 2 * k + 2, :],
                        wt[:, 2 * k:2 * k + 2, :],
                        start=(ti == 0 and k == 0),
                        stop=(ti == 2 and k == 3), perf_mode=DR)

            def proj_qk_tail(m):
                ps = proj_ps.pop(m)
                qkf = work.tile([128, 512], BF16, tag="qkf")
                nc.vector.tensor_copy(qkf[:], ps[:])
                if use_rope:
                    ccv = ropet[:, 0, m, :].unsqueeze(1).broadcast_to([128, 8, 64])
                    ssv = ropet[:, 1, m, :].unsqueeze(1).broadcast_to([128, 8, 64])
                    ta = work.tile([128, 512], BF16, tag="ta")
                    tav = ta.rearrange("p (h t e) -> p h t e", t=2, e=32)
                    tb = work.tile([128, 512], BF16, tag="tb")
                    tbv = tb.rearrange("p (h t e) -> p h t e", t=2, e=32)
                    nc.vector.tensor_mul(
                        ta.rearrange("p (h f) -> p h f", f=64),
                        qkf.rearrange("p (h f) -> p h f", f=64), ccv)
                    nc.vector.tensor_mul(
                        tb.rearrange("p (h f) -> p h f", f=64),
                        qkf.rearrange("p (h f) -> p h f", f=64), ssv)
                    qkr = work.tile([128, 512], BF16, tag="qkr")
                    qrv = qkr.rearrange("p (h t e) -> p h t e", t=2, e=32)
                    # outE = E*c - O*s ; outO = O*c + E*s
                    nc.gpsimd.tensor_sub(qrv[:, :, 0, :], tav[:, :, 0, :],
                                         tbv[:, :, 1, :])
                    nc.gpsimd.tensor_add(qrv[:, :, 1, :], tav[:, :, 1, :],
                                         tbv[:, :, 0, :])
                else:
                    qkr = qkf
                nc.sync.dma_start_transpose(qkt[m][:], qkr[:])

            def proj_v_term(m, ti):
                xt_t, moff = xt8s[m]
                if ti == 0:
                    if m < 2:
                        ps = scA.tile([128, 1024], F32, tag="scA",
                                      name=f"ppv{m}")[:, 0:256]
                    elif m < 6:
                        # early: the ypsum bank is mostly idle; keeps the V
                        # chain out of the shp slot so QK/V pipeline
                        ps = ypp.tile([128, HEADS_PER_CORE * 65], F32,
                                      tag="yp", name=f"ppv{m}")[:, 0:256]
                    else:
                        ps = shp.tile([128, 512], F32, tag="shp",
                                      name=f"ppv{m}")[:, 0:256]
                    projv_ps[m] = ps
                else:
                    ps = projv_ps[m]
                xo, wt = ((0, w8v), (0, e8v), (8, w8v))[ti]
                for k in range(4):
                    nc.tensor.matmul(
                        ps[:], xt_t[:, moff, xo + 2 * k:xo + 2 * k + 2, :],
                        wt[:, 2 * k:2 * k + 2, :],
                        start=(ti == 0 and k == 0),
                        stop=(ti == 2 and k == 3), perf_mode=DR)
                if ti == 2:
                    ps = projv_ps.pop(m)
                    dst = vsb[:, m, :].rearrange(
                        "p (h c) -> p h c", h=4)[:, :, 0:64]
                    src = ps.rearrange("p (h c) -> p h c", h=4)
                    nc.vector.tensor_copy(dst, src)

            def oproj_nb(m, nb, so):
                if m >= 13:  # attention is winding down; scores slots idle
                    po = scA.tile([128, 1024], F32, tag="scA",
                                  name=f"po{m}_{nb}")[:, 0:512]
                else:
                    po = shp.tile([128, 512], F32, tag="shp",
                                  name=f"po{m}_{nb}")
                for k in range(2):
                    nc.tensor.matmul(po[:], yt[m][:, k, :],
                                     wo[:, k, nb * 512:(nb + 1) * 512],
                                     start=(k == 0), stop=(k == 1))
                # high priority: the copy releases the single op psum bank,
                # which gates the next o_proj piece's matmuls on PE
                with tc.high_priority(offset=1000):
                    nc.vector.tensor_copy(so[:, nb * 512:(nb + 1) * 512], po[:])

            def proj_qk(m):
                proj_qk_term(m, 0)
                proj_qk_term(m, 1)
                proj_qk_term(m, 2)
                proj_qk_tail(m)

            def proj_v(m):
                proj_v_term(m, 0)
                proj_v_term(m, 1)
                proj_v_term(m, 2)

            # -------------- prologue: projections m=0,1 --------------
            # remaining projections staged across early (attention-light)
            # iterations: two per qb while qb<5, then one per qb
            proj_sched = {qb: [] for qb in range(SB)}
            for m in range(2):
                proj_qk(m)
                proj_v(m)
            for qb in range(5):
                proj_sched[qb] = [2 + 2 * qb, 3 + 2 * qb]
            for qb in range(5, 9):
                proj_sched[qb] = [12 + (qb - 5)]

            # o_proj work queue: (m, nb) pieces plus stores, consumed at a
            # lag of ~4 q-blocks so yt[m] transposes are long since done.
            oproj_work = []

            def oproj_piece():
                if not oproj_work:
                    return
                fn = oproj_work.pop(0)
                fn()

            so_tiles = {}
            opush_next = 0

            def push_oproj(m):
                def nb0():
                    so_tiles[m] = work.tile([128, D], BF16, tag="so",
                                            name=f"so{m}")
                    oproj_nb(m, 0, so_tiles[m])
                    if m >= 14:  # tail: don't gate the store on both halves
                        nc.sync.dma_start(out_d[m * 128:(m + 1) * 128, 0:512],
                                          so_tiles[m][:, 0:512])

                def nb1():
                    so = so_tiles.pop(m)
                    oproj_nb(m, 1, so)
                    if m >= 14:
                        nc.sync.dma_start(out_d[m * 128:(m + 1) * 128, 512:D],
                                          so[:, 512:D])
                    else:
                        nc.sync.dma_start(out_d[m * 128:(m + 1) * 128, :], so[:])

                oproj_work.append(nb0)
                oproj_work.append(nb1)

            # -------------- attention machinery (cross-qb pipelined) -------
            def chunks_of(qb):
                return [list(range(c0, min(c0 + CHUNK, qb + 1)))
                        for c0 in range(0, qb + 1, CHUNK)]

            pes = {}
            ypsums = {}
            sc_done = set()

            def scores(qb, hp, ci):
                # hi=0 blocks live in bank 0 (cols 0:512), hi=1 in bank 1
                # (cols 512:1024): matmuls with different PE row-group
                # tile_positions must not share a PSUM bank.
                if (qb, hp, ci) in sc_done:
                    return
                sc_done.add((qb, hp, ci))
                kbs = chunks_of(qb)[ci]
                w = len(kbs)
                sc = scA.tile([128, 1024], F32, tag="scA",
                              name=f"sc{qb}_{hp}_{ci}")
                for hi in range(2):
                    rows = slice(hi * 64, hi * 64 + 64)
                    for j, kb in enumerate(kbs):
                        nc.tensor.matmul(
                            sc[:, (hi * 4 + j) * 128:(hi * 4 + j + 1) * 128],
                            qkt[kb][rows, 2 + hp, :],
                            qkt[qb][rows, hp, :],
                            start=True, stop=True)
                pe_t = pex.tile([128, 1024], BF16, tag="pe",
                                name=f"pe{qb}_{hp}_{ci}")
                if w <= 2:  # two narrow exps beat spanning the bank gap
                    nc.scalar.activation(pe_t[:, 0:w * 128],
                                         sc[:, 0:w * 128], EXP, scale=ESCALE)
                    nc.scalar.activation(pe_t[:, 512:(4 + w) * 128],
                                         sc[:, 512:(4 + w) * 128], EXP,
                                         scale=ESCALE)
                else:
                    nc.scalar.activation(pe_t[:, 0:(4 + w) * 128],
                                         sc[:, 0:(4 + w) * 128], EXP,
                                         scale=ESCALE)
                if kbs[-1] == qb:  # causal diagonal block
                    for hi in range(2):
                        off = (hi * 4 + w - 1) * 128
                        nc.gpsimd.tensor_mul(
                            pe_t[:, off:off + 128],
                            pe_t[:, off:off + 128], maskT[:])
                pes[(qb, hp, ci)] = (pe_t, kbs)

            def pv_full(qb, hp):
                # One head's ENTIRE accumulation chain is contiguous: a PSUM
                # bank supports only one open accumulation group, so
                # interleaving two heads' start..stop chains in the same bank
                # corrupts the earlier one.
                if qb not in ypsums:
                    ypsums[qb] = ypp.tile([128, HEADS_PER_CORE * 65], F32,
                                          tag="yp", name=f"yp{qb}")
                ypsum = ypsums[qb]
                nch = len(chunks_of(qb))
                for hi in range(2):
                    h = 2 * hp + hi
                    vcol = slice(h * 65, h * 65 + 65)
                    for ci in range(nch):
                        pe_t, kbs = pes[(qb, hp, ci)]
                        for j, kb in enumerate(kbs):
                            nc.tensor.matmul(
                                ypsum[:, vcol],
                                pe_t[:, (hi * 4 + j) * 128:(hi * 4 + j + 1) * 128],
                                vsb[:, kb, vcol],
                                start=(kb == 0), stop=(kb == qb),
                                skip_group_check=True)
                for ci in range(nch):
                    pes.pop((qb, hp, ci))

            def normalize(qb):
                # high priority: frees the single ypsum bank for qb+1's PV
                ypsum = ypsums.pop(qb)
                yview = ypsum.rearrange("p (h c) -> p h c", c=65)
                with tc.high_priority(offset=1000):
                    rc = work.tile([128, 4], F32, tag="rc", name=f"rc{qb}")
                    nc.vector.reciprocal(rc[:], yview[:, :, 64])
                    ynorm = work.tile([128, 4, 64], BF16, tag="ynorm",
                                      name=f"yn{qb}")
                    nc.vector.tensor_mul(
                        ynorm[:], yview[:, :, 0:64],
                        rc.unsqueeze(2).broadcast_to([128, 4, 64]))
                if qb >= 14:
                    # tail: PE transpose (via idle scores slot) beats the
                    # ~2.4us XBAR DMA latency on the critical path
                    tp = scA.tile([128, 1024], F32, tag="scA",
                                  name=f"ytp{qb}").bitcast(BF16)
                    ynf = ynorm.rearrange("p h e -> p (h e)")
                    with tc.high_priority(offset=1000):
                        for b in range(2):
                            nc.tensor.transpose(
                                tp[:, b * 128:(b + 1) * 128],
                                ynf[:, b * 128:(b + 1) * 128], ident[:])
                        nc.vector.tensor_copy(
                            yt[qb][:], tp[:, 0:256].rearrange(
                                "p (a b) -> p a b", a=2))
                else:
                    nc.sync.dma_start_transpose(
                        yt[qb][:], ynorm.rearrange("p h e -> p (h e)"))

            # -------------- main loop over q-blocks --------------
            for qb in range(SB):
                nchunks = len(chunks_of(qb))
                # push o_proj work: lag 4 normally; from qb=13 push two per
                # iteration (lag >=1 on yt is safe) to drain the queue early
                target = qb - 4 if qb <= 12 else min(8 + 2 * (qb - 12), SB - 2)
                while opush_next <= target:
                    push_oproj(opush_next)
                    opush_next += 1
                npop = 2 if qb < 8 else 3

                pieces = []
                for m_ in proj_sched[qb]:
                    pieces.append(lambda m=m_: (proj_qk_term(m, 0),
                                                proj_qk_term(m, 1)))
                    pieces.append(lambda m=m_: (proj_qk_term(m, 2),
                                                proj_qk_tail(m)))
                    pieces.append(lambda m=m_: (proj_v_term(m, 0),
                                                proj_v_term(m, 1)))
                    pieces.append(lambda m=m_: proj_v_term(m, 2))
                for _ in range(npop):
                    pieces.append(oproj_piece)

                def filler(pieces=pieces):
                    if pieces:
                        pieces.pop(0)()

                for ci in range(nchunks):
                    scores(qb, 0, ci)   # may be a prefetched no-op
                    filler()
                scores(qb, 1, 0)
                filler()
                pv_full(qb, 0)
                for ci in range(1, nchunks):
                    scores(qb, 1, ci)
                    filler()
                while pieces:
                    filler()
                # prefetch next qb's first scores units to cover the
                # pv_full(1) + normalize latency at the iteration boundary
                if qb + 1 < SB:
                    scores(qb + 1, 0, 0)
                    if len(chunks_of(qb + 1)) > 1:
                        scores(qb + 1, 0, 1)
                pv_full(qb, 1)
                normalize(qb)

            # -------------- epilogue: drain remaining o_proj --------------
            push_oproj(SB - 1)
            while oproj_work:
                oproj_piece()

    nc.compile()
    return nc


def _perm_eo():
    """Per-head column permutation: [0,2,...,62, 1,3,...,63]."""
    return np.concatenate([np.arange(0, HD, 2), np.arange(1, HD, 2)])


def _prep_core_inputs(x, Wq, Wk, Wv, Wo, cos_g, sin_g, use_rope):
    maskT = np.tril(np.ones((128, 128), np.float32)).T.astype(_BF16)
    # rope tables tiled to [p, cc|ss, m, f] (f pattern = [cos|cos]/[sin|sin])
    cc = np.concatenate([cos_g, cos_g], axis=1)                  # [S, 64]
    ss = np.concatenate([sin_g, sin_g], axis=1)
    ropetab = np.stack([
        cc.reshape(SB, 128, 64).transpose(1, 0, 2),
        ss.reshape(SB, 128, 64).transpose(1, 0, 2)], axis=1).astype(_BF16)
    perm = _perm_eo()
    maps = []
    for c in range(NCORES):
        b, g = divmod(c, HEADS_PER_CORE)
        rows = slice(g * GDIM, (g + 1) * GDIM)
        wq = Wq[rows].reshape(HEADS_PER_CORE, HD, D)[:, perm, :].reshape(GDIM, D)
        wk = Wk[rows].reshape(HEADS_PER_CORE, HD, D)[:, perm, :].reshape(GDIM, D)
        w32 = np.concatenate([wq, wk, Wv[rows]], axis=0).T * WSCALE  # [D, 768]
        w8 = w32.astype(_FP8)
        e8 = (w32 - w8.astype(np.float32)).astype(_FP8)
        w8full = np.concatenate([w8, e8], axis=1)                   # [D, 1536]
        # xt8: [p, m, k, s'] with element (p,m,k,s') = x[b][m*128+s', k*128+p];
        # k planes 8-15 hold the fp8 residual of x
        xtl = x[b].T.reshape(8, 128, SB, 128).transpose(1, 2, 0, 3)
        x8 = xtl.astype(_FP8)
        r8 = (xtl - x8.astype(np.float32)).astype(_FP8)
        xt8 = np.concatenate([x8, r8], axis=2)       # [128, SB, 16, 128]
        maps.append({
            "xt8": np.ascontiguousarray(xt8),
            "w8": np.ascontiguousarray(w8full),
            "wo": np.ascontiguousarray(Wo[:, rows].T / WSCALE).astype(_BF16),
            "ropetab": ropetab,
            "maskT": maskT,
            "ident": np.eye(128, dtype=np.float32).astype(_BF16),
        })
    return maps


def kernel(x, token_positions, use_rope, Wq, Wk, Wv, Wo, cos, sin):
    from concourse.bass_utils import run_bass_kernel_spmd

    x = np.asarray(x, np.float32)
    token_positions = np.asarray(token_positions)
    Wq = np.asarray(Wq, np.float32)
    Wk = np.asarray(Wk, np.float32)
    Wv = np.asarray(Wv, np.float32)
    Wo = np.asarray(Wo, np.float32)
    cos = np.asarray(cos, np.float32)
    sin = np.asarray(sin, np.float32)
    rope = bool(int(use_rope))

    cos_g = cos[token_positions]  # [S, 32]
    sin_g = sin[token_positions]

    if rope not in _cache:
        _cache[rope] = _build(rope)
    nc = _cache[rope]

    in_maps = _prep_core_inputs(x, Wq, Wk, Wv, Wo, cos_g, sin_g, rope)
    res = run_bass_kernel_spmd(nc, in_maps, list(range(NCORES)))

    out = np.zeros((B, S, D), np.float32)
    for c in range(NCORES):
        out[c // HEADS_PER_CORE] += res.results[c]["out"].astype(np.float32)
    return out
